# revision 1
# baseline (speedup 1.0000x reference)
"""Trainium2 Bass kernel for nn_Exp_loss (exploded-logit / exponomial choice loss).

Math (per assortment row b, S=128 items, derived from the reference; see
kernel_v1 docstring for the step-by-step reduction): in DESCENDING-sorted
space d_0>=d_1>=... with P_i the inclusive prefix sum of d:
    TD_i = P_i - (i+1) d_i
    s    = sum relu(d - chosen)
    cnt  = #{k: d_k >= chosen} = i*+1;  term1 = 1/cnt
    inner = sum_i [d_i < chosen] * exp(min(s - TD_i, 0)) * wd_i,  wd_i = 1/(i(i+1))
    loss_b = log(term1 - inner) - s;  total = -sum_b loss_b / B

Distribution: pure data parallel, 256 rows/core across 8 cores; x is sharded
by item id per row on the host (gather + per-row id sort + one-hot chosen
extraction = pure index/layout work), final loss is a host-side all-reduce.

v3 engine plan:
  - Sort keys are bf16 (host-rounded; numerically validated at 9e-5 rel err
    vs the f32 reference, tolerance is 2e-2).  Halves both the sort-tile DMA
    and the DVE element traffic.
  - Each sort tile's load is split across the two HWDGE queues (SP +
    Activation engines) so the first tile (2x16KB) lands ~1us after issue.
  - DVE: the two 16-round max8/match_replace sorts (the scheduler interleaves
    them), then both tiles' post-chains (copy/scan/tt/stt/ts/recip) emitted
    step-interleaved so the chains hide each other's latency.
  - ACT: relu-accumulate (s), exp-accumulate (inner), one final Ln for both
    tiles (exactly two act-table loads, only the Ln one near the tail).
  - Pool: only trivial glue (memset/nch/argc/contrib) -- its ucode tensor ops
    are slow and share the DVE SBUF port.
"""

from contextlib import ExitStack

import numpy as np

import concourse.bass as bass
import concourse.bacc as bacc
import concourse.mybir as mybir
from concourse import tile
from concourse.bass_utils import run_bass_kernel_spmd

B, S = 2048, 128
N = B * S
N_CORES = 8
ROWS_PER_CORE = B // N_CORES          # 256
TILES_PER_CORE = ROWS_PER_CORE // 128  # 2
P = 128
HP = P // 2
NEG_BIG = -1.0e30   # match_replace filler: below any real score
MASK_BIG = 1.0e6    # added (negated) into exp arg to zero masked lanes

F32 = mybir.dt.float32
BF16 = mybir.dt.bfloat16
Alu = mybir.AluOpType
Act = mybir.ActivationFunctionType


def build_program():
    nc = bacc.Bacc()

    gx_d = nc.dram_tensor("gx", [P, TILES_PER_CORE * S], BF16, kind="ExternalInput")
    ch_d = nc.dram_tensor("ch", [P, TILES_PER_CORE], F32, kind="ExternalInput")
    # packed per-core constants: [:, 0:128] ln(wd), [:, 128:256] i+1
    consts_d = nc.dram_tensor("consts", [P, 2 * S], F32, kind="ExternalInput")
    out_d = nc.dram_tensor("partial", [P, TILES_PER_CORE], F32, kind="ExternalOutput")

    with tile.TileContext(nc) as tc, ExitStack() as ctx:
        const = ctx.enter_context(tc.tile_pool(name="const", bufs=1))
        big = ctx.enter_context(tc.tile_pool(name="big", bufs=4))
        work = ctx.enter_context(tc.tile_pool(name="work", bufs=20))
        cols = ctx.enter_context(tc.tile_pool(name="cols", bufs=16))
        fence_deps = []

        # Sort-tile loads, each split across both HWDGE queues by partition
        # half so the first tile lands as early as possible.
        gx_tiles = []
        for t in range(TILES_PER_CORE):
            g = big.tile([P, S], BF16, tag="gx")
            fence_deps.append(nc.sync.dma_start(
                g[0:HP, :], gx_d[0:HP, t * S:(t + 1) * S]))
            fence_deps.append(nc.scalar.dma_start(
                g[HP:P, :], gx_d[HP:P, t * S:(t + 1) * S]))
            gx_tiles.append(g)
        ch_sb = const.tile([P, TILES_PER_CORE], F32)
        fence_deps.append(nc.scalar.dma_start(ch_sb[:], ch_d[:]))
        consts_sb = const.tile([P, 2 * S], F32)
        fence_deps.append(nc.sync.dma_start(consts_sb[:], consts_d[:]))
        lnwd_sb = consts_sb[:, 0:S]
        ip1_sb = consts_sb[:, S:2 * S]

        zeros_sb = const.tile([P, S], F32)
        nc.gpsimd.memset(zeros_sb[:], 0.0)
        # negated chosen scores (bias for the relu(d - chosen) activation)
        nch = const.tile([P, TILES_PER_CORE], F32)
        nc.gpsimd.tensor_scalar(
            out=nch[:], in0=ch_sb[:], scalar1=-1.0, scalar2=None, op0=Alu.mult)

        s2 = const.tile([P, TILES_PER_CORE], F32)
        argc2 = const.tile([P, TILES_PER_CORE], F32)
        act_insts = []

        # ---- DVE: both sorts (scheduler interleaves the two dep chains) ----
        d_tiles = []
        for t in range(TILES_PER_CORE):
            g = gx_tiles[t]
            d = big.tile([P, S], BF16, tag="dsort")
            for k in range(S // 8):
                nc.vector.max(out=d[:, 8 * k:8 * k + 8], in_=g[:])
                if k != S // 8 - 1:
                    nc.vector.match_replace(
                        out=g[:], in_to_replace=d[:, 8 * k:8 * k + 8],
                        in_values=g[:], imm_value=NEG_BIG,
                    )
            d_tiles.append(d)

        # ---- post-chains on DVE/ACT, step-interleaved across tiles ----
        TT = TILES_PER_CORE
        d32 = [work.tile([P, S], F32, name=f"d32_{t}", tag=f"d32_{t}") for t in range(TT)]
        ps = [work.tile([P, S], F32, name=f"ps{t}", tag=f"ps{t}") for t in range(TT)]
        w1 = [work.tile([P, S], F32, name=f"w1{t}", tag=f"w1{t}") for t in range(TT)]
        ntd = [work.tile([P, S], F32, name=f"ntd{t}", tag=f"ntd{t}") for t in range(TT)]
        m = [work.tile([P, S], F32, name=f"m{t}", tag=f"m{t}") for t in range(TT)]
        fge = [work.tile([P, S], F32, name=f"fge{t}", tag=f"fge{t}") for t in range(TT)]
        mm = [work.tile([P, S], F32, name=f"mm{t}", tag=f"mm{t}") for t in range(TT)]
        junk = [work.tile([P, S], F32, name=f"junk{t}", tag=f"junk{t}") for t in range(TT)]
        e = [work.tile([P, S], F32, name=f"e{t}", tag=f"e{t}") for t in range(TT)]
        negs = [cols.tile([P, 1], F32, name=f"negs{t}", tag=f"negs{t}") for t in range(TT)]
        cnt1 = [cols.tile([P, 1], F32, name=f"cnt1{t}", tag=f"cnt1{t}") for t in range(TT)]
        inner = [cols.tile([P, 1], F32, name=f"inner{t}", tag=f"inner{t}") for t in range(TT)]
        t1 = [cols.tile([P, 1], F32, name=f"t1{t}", tag=f"t1{t}") for t in range(TT)]

        for t in range(TT):
            # bf16->f32 cast on ACT (Copy lives in every act table; keeps the
            # DVE queue free for the scan/tt chain that gates the tail)
            act_insts.append(nc.scalar.activation(
                out=d32[t][:], in_=d_tiles[t][:], func=Act.Copy))
        for t in range(TT):
            # s = sum relu(d - chosen) via the Relu accumulator (ACT engine)
            act_insts.append(nc.scalar.activation(
                out=junk[t][:], in_=d32[t][:], func=Act.Relu,
                bias=nch[:, t:t + 1], accum_out=s2[:, t:t + 1]))
        for t in range(TT):
            nc.vector.tensor_tensor_scan(
                out=ps[t][:], data0=d32[t][:], data1=zeros_sb[:], initial=0.0,
                op0=Alu.add, op1=Alu.add)
        for t in range(TT):
            nc.vector.tensor_tensor(
                out=w1[t][:], in0=d32[t][:], in1=ip1_sb, op=Alu.mult)
        for t in range(TT):
            nc.vector.tensor_tensor(
                out=ntd[t][:], in0=w1[t][:], in1=ps[t][:], op=Alu.subtract)
        for t in range(TT):
            nc.gpsimd.tensor_scalar(
                out=negs[t][:], in0=s2[:, t:t + 1], scalar1=-1.0, scalar2=None,
                op0=Alu.mult)
        for t in range(TT):
            # m = min(-TD, -s) + ln(wd)
            nc.vector.scalar_tensor_tensor(
                out=m[t][:], in0=ntd[t][:], scalar=negs[t][:], in1=lnwd_sb,
                op0=Alu.min, op1=Alu.add)
        for t in range(TT):
            # fge = [d >= chosen]; row count = i*+1 directly
            nc.vector.tensor_scalar(
                out=fge[t][:], in0=d32[t][:], scalar1=ch_sb[:, t:t + 1],
                scalar2=None, op0=Alu.is_ge, op1=Alu.add, accum_out=cnt1[t][:])
        for t in range(TT):
            # mm = m - MASK_BIG*fge: kept lanes unchanged, others -> -inf-ish
            nc.vector.scalar_tensor_tensor(
                out=mm[t][:], in0=fge[t][:], scalar=-MASK_BIG, in1=m[t][:],
                op0=Alu.mult, op1=Alu.add)
        for t in range(TT):
            # inner = sum exp(mm + s) via the Exp accumulator (ACT engine)
            act_insts.append(nc.scalar.activation(
                out=e[t][:], in_=mm[t][:], func=Act.Exp, bias=s2[:, t:t + 1],
                accum_out=inner[t][:]))
        for t in range(TT):
            nc.vector.reciprocal(out=t1[t][:], in_=cnt1[t][:])
        for t in range(TT):
            nc.gpsimd.tensor_tensor(
                out=argc2[:, t:t + 1], in0=t1[t][:], in1=inner[t][:],
                op=Alu.subtract)

        # single Ln over both tiles' args, contrib = ln - s, one output DMA
        ln2 = const.tile([P, TILES_PER_CORE], F32)
        act_insts.append(
            nc.scalar.activation(out=ln2[:], in_=argc2[:], func=Act.Ln))
        contrib2 = const.tile([P, TILES_PER_CORE], F32)
        nc.gpsimd.tensor_tensor(
            out=contrib2[:], in0=ln2[:], in1=s2[:], op=Alu.subtract)
        fence_deps.append(nc.sync.dma_start(out_d[:], contrib2[:]))

        # Staged SP fences: absorb per-proc completion sems a few at a time so
        # the kernel-tail Drain never carries more sync waits than the CTRL
        # instruction encoding allows.
        fence_deps.extend(act_insts[-2:])
        for i0 in range(0, len(fence_deps), 3):
            nop = nc.sync.nop()
            for dep in fence_deps[i0:i0 + 3]:
                tile.add_dep_helper(nop.ins, dep.ins, sync=True,
                                    reason="tail fence")

    nc.compile()
    return nc


def make_inputs(x, y, assortments):
    """Host-side sharding: per-core input maps (pure index/layout work)."""
    import ml_dtypes
    x = np.ascontiguousarray(np.asarray(x, dtype=np.float32).reshape(N))
    y = np.ascontiguousarray(np.asarray(y, dtype=np.float32).reshape(N))
    a = np.ascontiguousarray(np.asarray(assortments, dtype=np.int32).reshape(B, S))

    i = np.arange(S, dtype=np.float64)
    lnwd = np.full(S, -1.0e4, dtype=np.float32)
    lnwd[1:] = np.log(1.0 / (i[1:] * (i[1:] + 1.0))).astype(np.float32)
    consts = np.ascontiguousarray(np.tile(
        np.concatenate([lnwd, (i + 1.0).astype(np.float32)])[None, :], (P, 1)
    ).astype(np.float32))

    in_maps = []
    for c in range(N_CORES):
        rows = a[c * ROWS_PER_CORE:(c + 1) * ROWS_PER_CORE]  # [256, 128]
        rs = np.sort(rows, axis=1)  # per-row item ids ascending (id-order shard)
        xv16 = x[rs].astype(ml_dtypes.bfloat16)       # [256, S] bf16 sort keys
        cidx = np.argmax(y[rs], axis=1)               # one-hot position per row
        cv = xv16[np.arange(ROWS_PER_CORE), cidx].astype(np.float32)
        gx = np.ascontiguousarray(
            xv16.reshape(TILES_PER_CORE, P, S).transpose(1, 0, 2)
            .reshape(P, TILES_PER_CORE * S))
        ch = np.ascontiguousarray(cv.reshape(TILES_PER_CORE, P).T)
        in_maps.append({"gx": gx, "ch": ch, "consts": consts})
    return in_maps


_PROGRAM_CACHE = {}


def kernel(x, y, assortments, _want_trace=False, _trace_kwargs=None):
    assert np.asarray(x).size == N and np.asarray(assortments).shape == (B, S)
    in_maps = make_inputs(x, y, assortments)
    if "nc" not in _PROGRAM_CACHE:
        _PROGRAM_CACHE["nc"] = build_program()
    nc = _PROGRAM_CACHE["nc"]
    res = run_bass_kernel_spmd(
        nc, in_maps, core_ids=list(range(N_CORES)),
        trace=_want_trace, **(_trace_kwargs or {})
    )
    partials = [np.asarray(res.results[c]["partial"]).reshape(-1).sum(dtype=np.float64) for c in range(N_CORES)]
    total = np.float32(np.sum(np.stack(partials), dtype=np.float64))
    out = np.float32(-total / np.float32(B))
    if _want_trace:
        return out, res
    return out



# revision 2
# speedup vs baseline: 1.2109x; 1.2109x over previous
"""Trainium2 Bass kernel for nn_Exp_loss (exploded-logit / exponomial choice loss).

Math (per assortment row b, S=128 items): with d the DESCENDING-sorted scores,
P_i the inclusive prefix sum, TD_i = P_i - (i+1) d_i = sum_k relu(d_k - d_i),
s = sum_k relu(d_k - chosen) and wd_i = 1/(i(i+1)) (wd_0 := 0):

    raw    = sum_i exp(min(s - TD_i, 0) + ln wd_i)     # over ALL i
    loss_b = log(1 - raw) - s

This is exact: lanes with d_i >= chosen have TD_i <= s so they contribute
exactly wd_i, and sum_{i<=i*} wd_i telescopes to 1 - 1/cnt, which turns the
reference's  log(1/cnt - inner)  into  log(1 - raw)  with no mask / count /
reciprocal needed.

v4 engine plan:
  - Sort: 28-stage bitonic merge network (descending runs, merges pair run A
    with reversed run B via negative-stride APs) built from DVE tensor_tensor
    min/max ops in bf16.  Each stage is 2 ops that cover BOTH 128-row tiles
    (free size 128/op), ping-ponging between two buffers; the DMA-target
    buffer A is never overwritten so the ACT engine can accumulate
    s = sum relu(x - chosen) from the raw input concurrently with the sort.
  - Post (mostly FD=256, batched across the two tiles): ACT cast -> DVE
    prefix-sum scans (per tile) || Pool w1 = d*(i+1) -> DVE ntd = w1 - ps
    -> DVE per-tile m' = min(ntd + s, 0) -> DVE q = m' + ln wd -> ACT exp
    with row-accumulator (raw) -> ACT ln(1 - raw) via scale=-1/bias=1
    -> Pool contrib = ln - s -> one output DMA.

Distribution: pure data parallel, 256 rows/core across 8 cores; the host does
index/layout work only (gather by assortment ids + bf16 cast + one-hot chosen
extraction), final loss is a host-side all-reduce of per-row contributions.
"""

from contextlib import ExitStack

import numpy as np

import concourse.bass as bass
import concourse.bacc as bacc
import concourse.mybir as mybir
from concourse import tile
from concourse.bass_utils import run_bass_kernel_spmd

B, S = 2048, 128
N = B * S
N_CORES = 8
ROWS_PER_CORE = B // N_CORES          # 256
TILES_PER_CORE = ROWS_PER_CORE // 128  # 2
P = 128
W = TILES_PER_CORE * S                 # 256 columns (both tiles)

F32 = mybir.dt.float32
BF16 = mybir.dt.bfloat16
Alu = mybir.AluOpType
Act = mybir.ActivationFunctionType


def build_program():
    nc = bacc.Bacc()

    gx_d = nc.dram_tensor("gx", [P, W], BF16, kind="ExternalInput")
    # packed per-core constants: [:, 0:256] ln(wd) x2, [:, 256:512] (i+1) x2,
    # [:, 512:514] negated chosen scores per tile
    consts_d = nc.dram_tensor("consts", [P, 2 * W + TILES_PER_CORE], F32,
                              kind="ExternalInput")
    out_d = nc.dram_tensor("partial", [P, TILES_PER_CORE], F32,
                           kind="ExternalOutput")

    with tile.TileContext(nc) as tc, ExitStack() as ctx:
        const = ctx.enter_context(tc.tile_pool(name="const", bufs=1))
        big = ctx.enter_context(tc.tile_pool(name="big", bufs=3))
        work = ctx.enter_context(tc.tile_pool(name="work", bufs=12))
        cols = ctx.enter_context(tc.tile_pool(name="cols", bufs=4))
        fence_deps = []

        # ---- input DMAs, split across the two HWDGE queues ----
        A = big.tile([P, W], BF16, tag="A")
        fence_deps.append(nc.sync.dma_start(A[0:64, :], gx_d[0:64, :]))
        fence_deps.append(nc.scalar.dma_start(A[64:P, :], gx_d[64:P, :]))
        consts_sb = const.tile([P, 2 * W + TILES_PER_CORE], F32)
        fence_deps.append(nc.sync.dma_start(consts_sb[:], consts_d[:]))
        lnwd2 = consts_sb[:, 0:W]
        ip2 = consts_sb[:, W:2 * W]
        nch = consts_sb[:, 2 * W:2 * W + TILES_PER_CORE]

        zeros_sb = const.tile([P, S], F32)
        nc.gpsimd.memset(zeros_sb[:], 0.0)

        s2 = const.tile([P, TILES_PER_CORE], F32)
        raw2 = const.tile([P, TILES_PER_CORE], F32)
        act_insts = []

        # ---- s accumulation from the RAW (unsorted) input, overlapped with
        # the sort: s = sum relu(x - chosen) is order-independent.
        junk = work.tile([P, W], BF16, name="junk", tag="junk")
        for t in range(TILES_PER_CORE):
            act_insts.append(nc.scalar.activation(
                out=junk[:, t * S:(t + 1) * S], in_=A[:, t * S:(t + 1) * S],
                func=Act.Relu, bias=nch[:, t:t + 1],
                accum_out=s2[:, t:t + 1]))

        # ---- bitonic sort network: 28 stages x 2 DVE tensor_tensor ops ----
        Bt = big.tile([P, W], BF16, tag="B")
        Ct = big.tile([P, W], BF16, tag="C")
        pingpong = [Bt, Ct]
        k = 0
        src = A
        for L in (1, 2, 4, 8, 16, 32, 64):
            # merge stage: pair run A[i] with reversed run B (cols 2L-1-i)
            dst = pingpong[k % 2]
            k += 1
            nbt = S // (2 * L)
            if nbt > 1:
                vs = src[:].rearrange("p (t nb c) -> p t nb c",
                                      t=TILES_PER_CORE, nb=nbt, c=2 * L)
                vd = dst[:].rearrange("p (t nb c) -> p t nb c",
                                      t=TILES_PER_CORE, nb=nbt, c=2 * L)
                lo_i = vs[:, :, :, 0:L]
                hirev_i = vs[:, :, :, 2 * L - 1:L - 1:-1]
                lo_o = vd[:, :, :, 0:L]
                lorev_i = vs[:, :, :, L - 1::-1]
                hi_i = vs[:, :, :, L:2 * L]
                hi_o = vd[:, :, :, L:2 * L]
            else:
                vs = src[:].rearrange("p (nb c) -> p nb c",
                                      nb=W // (2 * L), c=2 * L)
                vd = dst[:].rearrange("p (nb c) -> p nb c",
                                      nb=W // (2 * L), c=2 * L)
                lo_i = vs[:, :, 0:L]
                hirev_i = vs[:, :, 2 * L - 1:L - 1:-1]
                lo_o = vd[:, :, 0:L]
                lorev_i = vs[:, :, L - 1::-1]
                hi_i = vs[:, :, L:2 * L]
                hi_o = vd[:, :, L:2 * L]
            nc.vector.tensor_tensor(out=lo_o, in0=lo_i, in1=hirev_i,
                                    op=Alu.max)
            nc.vector.tensor_tensor(out=hi_o, in0=lorev_i, in1=hi_i,
                                    op=Alu.min)
            src = dst
            d = L // 2
            while d >= 1:
                dst = pingpong[k % 2]
                k += 1
                vs = src[:].rearrange("p (nb c) -> p nb c",
                                      nb=W // (2 * d), c=2 * d)
                vd = dst[:].rearrange("p (nb c) -> p nb c",
                                      nb=W // (2 * d), c=2 * d)
                nc.vector.tensor_tensor(out=vd[:, :, 0:d], in0=vs[:, :, 0:d],
                                        in1=vs[:, :, d:2 * d], op=Alu.max)
                nc.vector.tensor_tensor(out=vd[:, :, d:2 * d],
                                        in0=vs[:, :, 0:d],
                                        in1=vs[:, :, d:2 * d], op=Alu.min)
                src = dst
                d //= 2
        D = src  # descending-sorted bf16, both tiles

        # ---- post-chain ----
        d32 = work.tile([P, W], F32, name="d32", tag="d32")
        act_insts.append(nc.scalar.activation(out=d32[:], in_=D[:],
                                              func=Act.Copy))
        ps2 = work.tile([P, W], F32, name="ps2", tag="ps2")
        for t in range(TILES_PER_CORE):
            nc.vector.tensor_tensor_scan(
                out=ps2[:, t * S:(t + 1) * S], data0=d32[:, t * S:(t + 1) * S],
                data1=zeros_sb[:], initial=0.0, op0=Alu.add, op1=Alu.add)
        w1 = work.tile([P, W], F32, name="w1", tag="w1")
        nc.gpsimd.tensor_tensor(out=w1[:], in0=d32[:], in1=ip2, op=Alu.mult)
        ntd = work.tile([P, W], F32, name="ntd", tag="ntd")
        nc.vector.tensor_tensor(out=ntd[:], in0=w1[:], in1=ps2[:],
                                op=Alu.subtract)
        # per tile: m' = min(ntd + s, 0); q = m' + ln wd; raw = sum exp(q)
        mprime = work.tile([P, W], F32, name="mprime", tag="mprime")
        q2 = work.tile([P, W], F32, name="q2", tag="q2")
        e2 = work.tile([P, W], F32, name="e2", tag="e2")
        for t in range(TILES_PER_CORE):
            sl = slice(t * S, (t + 1) * S)
            nc.vector.scalar_tensor_tensor(
                out=mprime[:, sl], in0=ntd[:, sl], scalar=s2[:, t:t + 1],
                in1=zeros_sb[:], op0=Alu.add, op1=Alu.min)
            nc.vector.tensor_tensor(
                out=q2[:, sl], in0=mprime[:, sl], in1=lnwd2[:, sl],
                op=Alu.add)
            act_insts.append(nc.scalar.activation(
                out=e2[:, sl], in_=q2[:, sl], func=Act.Exp,
                accum_out=raw2[:, t:t + 1]))

        # ln(1 - raw) in one ACT op, contrib = ln - s, one output DMA
        ln2 = cols.tile([P, TILES_PER_CORE], F32, name="ln2", tag="ln2")
        act_insts.append(nc.scalar.activation(
            out=ln2[:], in_=raw2[:], func=Act.Ln, scale=-1.0, bias=1.0))
        contrib2 = cols.tile([P, TILES_PER_CORE], F32, name="contrib2",
                             tag="contrib2")
        nc.gpsimd.tensor_tensor(out=contrib2[:], in0=ln2[:], in1=s2[:],
                                op=Alu.subtract)
        fence_deps.append(nc.sync.dma_start(out_d[:], contrib2[:]))

        # Staged SP fences: absorb per-proc completion sems a few at a time so
        # the kernel-tail Drain never carries more sync waits than the CTRL
        # instruction encoding allows.
        fence_deps.extend(act_insts[-2:])
        for i0 in range(0, len(fence_deps), 3):
            nop = nc.sync.nop()
            for dep in fence_deps[i0:i0 + 3]:
                tile.add_dep_helper(nop.ins, dep.ins, sync=True,
                                    reason="tail fence")

    nc.compile()
    return nc


def make_inputs(x, y, assortments):
    """Host-side sharding: per-core input maps (pure index/layout work)."""
    import ml_dtypes
    x = np.ascontiguousarray(np.asarray(x, dtype=np.float32).reshape(N))
    y = np.ascontiguousarray(np.asarray(y, dtype=np.float32).reshape(N))
    a = np.ascontiguousarray(np.asarray(assortments, dtype=np.int32).reshape(B, S))

    i = np.arange(S, dtype=np.float64)
    lnwd = np.full(S, -1.0e4, dtype=np.float32)
    lnwd[1:] = np.log(1.0 / (i[1:] * (i[1:] + 1.0))).astype(np.float32)
    lnwd2 = np.tile(lnwd, TILES_PER_CORE)
    ip2 = np.tile((i + 1.0).astype(np.float32), TILES_PER_CORE)

    in_maps = []
    for c in range(N_CORES):
        rows = a[c * ROWS_PER_CORE:(c + 1) * ROWS_PER_CORE]  # [256, 128]
        rs = np.sort(rows, axis=1)  # per-row item ids ascending (id-order shard)
        xv16 = x[rs].astype(ml_dtypes.bfloat16)       # [256, S] bf16 scores
        cidx = np.argmax(y[rs], axis=1)               # one-hot position per row
        cv = xv16[np.arange(ROWS_PER_CORE), cidx].astype(np.float32)
        gx = np.ascontiguousarray(
            xv16.reshape(TILES_PER_CORE, P, S).transpose(1, 0, 2)
            .reshape(P, W))
        nch = np.ascontiguousarray(-cv.reshape(TILES_PER_CORE, P).T)
        consts = np.ascontiguousarray(np.concatenate([
            np.tile(lnwd2[None, :], (P, 1)),
            np.tile(ip2[None, :], (P, 1)),
            nch,
        ], axis=1).astype(np.float32))
        in_maps.append({"gx": gx, "consts": consts})
    return in_maps


_PROGRAM_CACHE = {}


def kernel(x, y, assortments, _want_trace=False, _trace_kwargs=None):
    assert np.asarray(x).size == N and np.asarray(assortments).shape == (B, S)
    in_maps = make_inputs(x, y, assortments)
    if "nc" not in _PROGRAM_CACHE:
        _PROGRAM_CACHE["nc"] = build_program()
    nc = _PROGRAM_CACHE["nc"]
    res = run_bass_kernel_spmd(
        nc, in_maps, core_ids=list(range(N_CORES)),
        trace=_want_trace, **(_trace_kwargs or {})
    )
    partials = [np.asarray(res.results[c]["partial"]).reshape(-1).sum(dtype=np.float64) for c in range(N_CORES)]
    total = np.float32(np.sum(np.stack(partials), dtype=np.float64))
    out = np.float32(-total / np.float32(B))
    if _want_trace:
        return out, res
    return out


# revision 3
# speedup vs baseline: 1.2418x; 1.0255x over previous
"""Trainium2 Bass kernel for nn_Exp_loss (exploded-logit / exponomial choice loss).

Math (per assortment row b, S=128 items): with d the DESCENDING-sorted scores,
P_i the inclusive prefix sum, TD_i = P_i - (i+1) d_i = sum_k relu(d_k - d_i),
s = sum_k relu(d_k - chosen) and wd_i = 1/(i(i+1)) (wd_0 := 0):

    raw    = sum_i exp(min(s - TD_i, 0) + ln wd_i)     # over ALL i
    loss_b = log(1 - raw) - s

This is exact: lanes with d_i >= chosen have TD_i <= s so they contribute
exactly wd_i, and sum_{i<=i*} wd_i telescopes to 1 - 1/cnt, which turns the
reference's  log(1/cnt - inner)  into  log(1 - raw)  with no mask / count /
reciprocal needed.

v5 engine plan:
  - Sort: 28-stage bitonic merge network (descending runs, merges pair run A
    with reversed run B via negative-stride APs) built from DVE tensor_tensor
    min/max ops in bf16.  Each stage is 2 ops covering BOTH 128-row tiles
    (free size 128/op), ping-ponging between two buffers; the DMA-target
    buffer A stays live so the ACT engine accumulates s = sum relu(x - ch)
    from the raw input concurrently with the sort.
  - Post, no f32 cast anywhere: one fused prefix-sum scan over [P,256]
    (bf16 in -> f32 out) with the tile-1 boundary folded into the per-tile
    min() scalar; w1 = d*(i+1) on Pool from bf16 inputs; ntd = w1 - ps;
    m' = (ntd + s) min 0 via tensor_scalar (2x_2p); q = m' + ln wd;
    ACT exp with row accumulator -> ln(1-raw) via scale=-1/bias=1 ->
    Pool contrib = ln - s -> one output DMA.
  - All ACT funcs (Relu/Exp/Ln) are pinned to the combined
    natural_log_exp_and_others table so the single ACT_TABLE_LOAD happens
    at kernel start instead of a second load serializing the tail.
  - A dummy 8-wide scan runs during the input-DMA window to absorb the
    first-TTS ucode warmup penalty (~570ns) off the critical path.
"""

from contextlib import ExitStack

import numpy as np

import concourse.bass as bass
import concourse.bacc as bacc
import concourse.mybir as mybir
from concourse import tile
from concourse.bass_utils import run_bass_kernel_spmd

B, S = 2048, 128
N = B * S
N_CORES = 8
ROWS_PER_CORE = B // N_CORES          # 256
TILES_PER_CORE = ROWS_PER_CORE // 128  # 2
P = 128
W = TILES_PER_CORE * S                 # 256 columns (both tiles)

F32 = mybir.dt.float32
BF16 = mybir.dt.bfloat16
Alu = mybir.AluOpType
Act = mybir.ActivationFunctionType

_ACT_TABLE_PATCHED = False


def _patch_act_tables():
    """Prefer the combined exp+ln activation table.

    The table-load pass assigns each activation the first table containing
    its function, which puts Exp in `exp_and_others` and Ln in `natural_log`
    — forcing a second ~1.3us ACT_TABLE_LOAD right before the final Ln on
    the kernel's critical tail.  Emptying the sets that are subsets of
    `natural_log_exp_and_others` (same names/indices kept, so the emitted
    act_func_set_id stays a valid act_info.json index) makes every function
    resolve to the combined table: one load, hoisted to kernel start.
    """
    global _ACT_TABLE_PATCHED
    if _ACT_TABLE_PATCHED:
        return
    import concourse.bacc as bacc_mod
    from concourse.hw_specs import get_activation_tables as _orig
    target = "natural_log_exp_and_others"

    def patched(arch):
        tabs = _orig(arch)
        if target not in tabs:
            return tabs
        big = tabs[target]
        return {
            name: (funcs if (name == target or not (funcs <= big)) else set())
            for name, funcs in tabs.items()
        }

    bacc_mod.get_activation_tables = patched
    _ACT_TABLE_PATCHED = True


def build_program():
    _patch_act_tables()
    nc = bacc.Bacc()

    gx_d = nc.dram_tensor("gx", [P, W], BF16, kind="ExternalInput")
    # packed per-core constants (f32 words): [0:256] ln(wd) x2,
    # [256:384] (i+1) x2 as packed bf16 pairs, [384:386] negated chosen
    consts_d = nc.dram_tensor("consts", [P, W + S + TILES_PER_CORE], F32,
                              kind="ExternalInput")
    out_d = nc.dram_tensor("partial", [P, TILES_PER_CORE], F32,
                           kind="ExternalOutput")

    with tile.TileContext(nc) as tc, ExitStack() as ctx:
        const = ctx.enter_context(tc.tile_pool(name="const", bufs=1))
        big = ctx.enter_context(tc.tile_pool(name="big", bufs=3))
        work = ctx.enter_context(tc.tile_pool(name="work", bufs=12))
        cols = ctx.enter_context(tc.tile_pool(name="cols", bufs=6))
        fence_deps = []

        # ---- input DMAs, split across the two HWDGE queues ----
        A = big.tile([P, W], BF16, tag="A")
        fence_deps.append(nc.sync.dma_start(A[0:64, :], gx_d[0:64, :]))
        fence_deps.append(nc.scalar.dma_start(A[64:P, :], gx_d[64:P, :]))
        consts_sb = const.tile([P, W + S + TILES_PER_CORE], F32)
        fence_deps.append(nc.sync.dma_start(consts_sb[:], consts_d[:]))
        lnwd2 = consts_sb[:, 0:W]
        ip2 = consts_sb[:, W:W + S].bitcast(BF16)        # [P, 256] bf16
        nch = consts_sb[:, W + S:W + S + TILES_PER_CORE]

        zeros16 = const.tile([P, W], BF16)
        nc.gpsimd.memset(zeros16[:], 0.0)

        # dummy scan: absorbs the first-TTS warmup penalty during DMA wait
        dummy = cols.tile([P, 8], F32, name="dummy", tag="dummy")
        nc.vector.tensor_tensor_scan(
            out=dummy[:], data0=zeros16[:, 0:8], data1=zeros16[:, 0:8],
            initial=0.0, op0=Alu.add, op1=Alu.add)

        s2 = const.tile([P, TILES_PER_CORE], F32)
        raw2 = const.tile([P, TILES_PER_CORE], F32)
        act_insts = []

        # ---- s accumulation from the RAW (unsorted) input, overlapped with
        # the sort: s = sum relu(x - chosen) is order-independent.
        junk = work.tile([P, W], BF16, name="junk", tag="junk")
        for t in range(TILES_PER_CORE):
            act_insts.append(nc.scalar.activation(
                out=junk[:, t * S:(t + 1) * S], in_=A[:, t * S:(t + 1) * S],
                func=Act.Relu, bias=nch[:, t:t + 1],
                accum_out=s2[:, t:t + 1]))

        # ---- bitonic sort network: 28 stages x 2 DVE tensor_tensor ops ----
        Bt = big.tile([P, W], BF16, tag="B")
        Ct = big.tile([P, W], BF16, tag="C")
        pingpong = [Bt, Ct]
        k = 0
        src = A
        for L in (1, 2, 4, 8, 16, 32, 64):
            # merge stage: pair run A[i] with reversed run B (cols 2L-1-i)
            dst = pingpong[k % 2]
            k += 1
            nbt = S // (2 * L)
            if nbt > 1:
                vs = src[:].rearrange("p (t nb c) -> p t nb c",
                                      t=TILES_PER_CORE, nb=nbt, c=2 * L)
                vd = dst[:].rearrange("p (t nb c) -> p t nb c",
                                      t=TILES_PER_CORE, nb=nbt, c=2 * L)
                lo_i = vs[:, :, :, 0:L]
                hirev_i = vs[:, :, :, 2 * L - 1:L - 1:-1]
                lo_o = vd[:, :, :, 0:L]
                lorev_i = vs[:, :, :, L - 1::-1]
                hi_i = vs[:, :, :, L:2 * L]
                hi_o = vd[:, :, :, L:2 * L]
            else:
                vs = src[:].rearrange("p (nb c) -> p nb c",
                                      nb=W // (2 * L), c=2 * L)
                vd = dst[:].rearrange("p (nb c) -> p nb c",
                                      nb=W // (2 * L), c=2 * L)
                lo_i = vs[:, :, 0:L]
                hirev_i = vs[:, :, 2 * L - 1:L - 1:-1]
                lo_o = vd[:, :, 0:L]
                lorev_i = vs[:, :, L - 1::-1]
                hi_i = vs[:, :, L:2 * L]
                hi_o = vd[:, :, L:2 * L]
            nc.vector.tensor_tensor(out=lo_o, in0=lo_i, in1=hirev_i,
                                    op=Alu.max)
            nc.vector.tensor_tensor(out=hi_o, in0=lorev_i, in1=hi_i,
                                    op=Alu.min)
            src = dst
            d = L // 2
            while d >= 1:
                dst = pingpong[k % 2]
                k += 1
                vs = src[:].rearrange("p (nb c) -> p nb c",
                                      nb=W // (2 * d), c=2 * d)
                vd = dst[:].rearrange("p (nb c) -> p nb c",
                                      nb=W // (2 * d), c=2 * d)
                nc.vector.tensor_tensor(out=vd[:, :, 0:d], in0=vs[:, :, 0:d],
                                        in1=vs[:, :, d:2 * d], op=Alu.max)
                nc.vector.tensor_tensor(out=vd[:, :, d:2 * d],
                                        in0=vs[:, :, 0:d],
                                        in1=vs[:, :, d:2 * d], op=Alu.min)
                src = dst
                d //= 2
        D = src  # descending-sorted bf16, both tiles

        # ---- post-chain (no f32 cast; bf16 ins -> f32 outs) ----
        # fused prefix sum across both tiles; tile1's offset is corrected
        # via the per-partition scalar folded into its min() op below
        ps = work.tile([P, W], F32, name="ps", tag="ps")
        nc.vector.tensor_tensor_scan(
            out=ps[:], data0=D[:], data1=zeros16[:], initial=0.0,
            op0=Alu.add, op1=Alu.add)
        # sc1 = s2[tile1] + ps[:, 127]  (boundary correction + s in one)
        sc1 = cols.tile([P, 1], F32, name="sc1", tag="sc1")
        nc.gpsimd.tensor_tensor(out=sc1[:], in0=s2[:, 1:2],
                                in1=ps[:, S - 1:S], op=Alu.add)
        w1 = work.tile([P, W], F32, name="w1", tag="w1")
        nc.gpsimd.tensor_tensor(out=w1[:], in0=D[:], in1=ip2, op=Alu.mult)
        ntd = work.tile([P, W], F32, name="ntd", tag="ntd")
        nc.vector.tensor_tensor(out=ntd[:], in0=w1[:], in1=ps[:],
                                op=Alu.subtract)
        # per tile: m' = min(ntd + s, 0); q = m' + ln wd; raw = sum exp(q)
        mprime = work.tile([P, W], F32, name="mprime", tag="mprime")
        q2 = work.tile([P, W], F32, name="q2", tag="q2")
        e2 = work.tile([P, W], F32, name="e2", tag="e2")
        svec = [s2[:, 0:1], sc1[:]]
        for t in range(TILES_PER_CORE):
            sl = slice(t * S, (t + 1) * S)
            nc.vector.tensor_scalar(
                out=mprime[:, sl], in0=ntd[:, sl], scalar1=svec[t],
                scalar2=0.0, op0=Alu.add, op1=Alu.min)
            nc.vector.tensor_tensor(
                out=q2[:, sl], in0=mprime[:, sl], in1=lnwd2[:, sl],
                op=Alu.add)
            act_insts.append(nc.scalar.activation(
                out=e2[:, sl], in_=q2[:, sl], func=Act.Exp,
                accum_out=raw2[:, t:t + 1]))

        # ln(1 - raw) in one ACT op, contrib = ln - s, one output DMA
        ln2 = cols.tile([P, TILES_PER_CORE], F32, name="ln2", tag="ln2")
        act_insts.append(nc.scalar.activation(
            out=ln2[:], in_=raw2[:], func=Act.Ln, scale=-1.0, bias=1.0))
        contrib2 = cols.tile([P, TILES_PER_CORE], F32, name="contrib2",
                             tag="contrib2")
        nc.gpsimd.tensor_tensor(out=contrib2[:], in0=ln2[:], in1=s2[:],
                                op=Alu.subtract)
        fence_deps.append(nc.sync.dma_start(out_d[:], contrib2[:]))

        # Staged SP fences: absorb per-proc completion sems a few at a time so
        # the kernel-tail Drain never carries more sync waits than the CTRL
        # instruction encoding allows.
        fence_deps.extend(act_insts[-2:])
        for i0 in range(0, len(fence_deps), 3):
            nop = nc.sync.nop()
            for dep in fence_deps[i0:i0 + 3]:
                tile.add_dep_helper(nop.ins, dep.ins, sync=True,
                                    reason="tail fence")

    nc.compile()
    return nc


def make_inputs(x, y, assortments):
    """Host-side sharding: per-core input maps (pure index/layout work)."""
    import ml_dtypes
    x = np.ascontiguousarray(np.asarray(x, dtype=np.float32).reshape(N))
    y = np.ascontiguousarray(np.asarray(y, dtype=np.float32).reshape(N))
    a = np.ascontiguousarray(np.asarray(assortments, dtype=np.int32).reshape(B, S))

    i = np.arange(S, dtype=np.float64)
    lnwd = np.full(S, -1.0e4, dtype=np.float32)
    lnwd[1:] = np.log(1.0 / (i[1:] * (i[1:] + 1.0))).astype(np.float32)
    lnwd2 = np.tile(lnwd, TILES_PER_CORE)
    ip2_words = np.ascontiguousarray(
        np.tile((i + 1.0).astype(ml_dtypes.bfloat16), TILES_PER_CORE)
    ).view(np.float32)  # 256 bf16 -> 128 f32 words

    in_maps = []
    for c in range(N_CORES):
        rows = a[c * ROWS_PER_CORE:(c + 1) * ROWS_PER_CORE]  # [256, 128]
        rs = np.sort(rows, axis=1)  # per-row item ids ascending (id-order shard)
        xv16 = x[rs].astype(ml_dtypes.bfloat16)       # [256, S] bf16 scores
        cidx = np.argmax(y[rs], axis=1)               # one-hot position per row
        cv = xv16[np.arange(ROWS_PER_CORE), cidx].astype(np.float32)
        gx = np.ascontiguousarray(
            xv16.reshape(TILES_PER_CORE, P, S).transpose(1, 0, 2)
            .reshape(P, W))
        nch = np.ascontiguousarray(-cv.reshape(TILES_PER_CORE, P).T)
        consts = np.ascontiguousarray(np.concatenate([
            np.tile(lnwd2[None, :], (P, 1)),
            np.tile(ip2_words[None, :], (P, 1)),
            nch,
        ], axis=1).astype(np.float32))
        in_maps.append({"gx": gx, "consts": consts})
    return in_maps


_PROGRAM_CACHE = {}


def kernel(x, y, assortments, _want_trace=False, _trace_kwargs=None):
    assert np.asarray(x).size == N and np.asarray(assortments).shape == (B, S)
    in_maps = make_inputs(x, y, assortments)
    if "nc" not in _PROGRAM_CACHE:
        _PROGRAM_CACHE["nc"] = build_program()
    nc = _PROGRAM_CACHE["nc"]
    res = run_bass_kernel_spmd(
        nc, in_maps, core_ids=list(range(N_CORES)),
        trace=_want_trace, **(_trace_kwargs or {})
    )
    partials = [np.asarray(res.results[c]["partial"]).reshape(-1).sum(dtype=np.float64) for c in range(N_CORES)]
    total = np.float32(np.sum(np.stack(partials), dtype=np.float64))
    out = np.float32(-total / np.float32(B))
    if _want_trace:
        return out, res
    return out


# revision 5
# speedup vs baseline: 1.3242x; 1.0664x over previous
"""Trainium2 Bass kernel for nn_Exp_loss (exploded-logit / exponomial choice loss).

Math (per assortment row b, S=128 items): with d the DESCENDING-sorted scores,
P_i the inclusive prefix sum, TD_i = P_i - (i+1) d_i = sum_k relu(d_k - d_i),
s = sum_k relu(d_k - chosen) and wd_i = 1/(i(i+1)) (wd_0 := 0):

    raw    = sum_i exp(min(s - TD_i, 0) + ln wd_i)     # over ALL i
    loss_b = log(1 - raw) - s

This is exact: lanes with d_i >= chosen have TD_i <= s so they contribute
exactly wd_i, and sum_{i<=i*} wd_i telescopes to 1 - 1/cnt, which turns the
reference's  log(1/cnt - inner)  into  log(1 - raw)  with no mask / count /
reciprocal needed.

v5 engine plan:
  - Sort: 28-stage bitonic merge network (descending runs, merges pair run A
    with reversed run B via negative-stride APs) built from DVE tensor_tensor
    min/max ops in bf16.  Each stage is 2 ops covering BOTH 128-row tiles
    (free size 128/op), ping-ponging between two buffers; the DMA-target
    buffer A stays live so the ACT engine accumulates s = sum relu(x - ch)
    from the raw input concurrently with the sort.
  - Post, no f32 cast anywhere: one fused prefix-sum scan over [P,256]
    (bf16 in -> f32 out) with the tile-1 boundary folded into the per-tile
    min() scalar; w1 = d*(i+1) on Pool from bf16 inputs; ntd = w1 - ps;
    m' = (ntd + s) min 0 via tensor_scalar (2x_2p); q = m' + ln wd;
    ACT exp with row accumulator -> ln(1-raw) via scale=-1/bias=1 ->
    Pool contrib = ln - s -> one output DMA.
  - All ACT funcs (Relu/Exp/Ln) are pinned to the combined
    natural_log_exp_and_others table so the single ACT_TABLE_LOAD happens
    at kernel start instead of a second load serializing the tail.
  - A dummy 8-wide scan runs during the input-DMA window to absorb the
    first-TTS ucode warmup penalty (~570ns) off the critical path.
"""

from contextlib import ExitStack

import numpy as np

import concourse.bass as bass
import concourse.bacc as bacc
import concourse.mybir as mybir
from concourse import tile
from concourse.bass_utils import run_bass_kernel_spmd

B, S = 2048, 128
N = B * S
N_CORES = 8
ROWS_PER_CORE = B // N_CORES          # 256
TILES_PER_CORE = ROWS_PER_CORE // 128  # 2
P = 128
W = TILES_PER_CORE * S                 # 256 columns (both tiles)

F32 = mybir.dt.float32
BF16 = mybir.dt.bfloat16
Alu = mybir.AluOpType
Act = mybir.ActivationFunctionType

_ACT_TABLE_PATCHED = False


def _patch_act_tables():
    """Prefer the combined exp+ln activation table.

    The table-load pass assigns each activation the first table containing
    its function, which puts Exp in `exp_and_others` and Ln in `natural_log`
    — forcing a second ~1.3us ACT_TABLE_LOAD right before the final Ln on
    the kernel's critical tail.  Emptying the sets that are subsets of
    `natural_log_exp_and_others` (same names/indices kept, so the emitted
    act_func_set_id stays a valid act_info.json index) makes every function
    resolve to the combined table: one load, hoisted to kernel start.
    """
    global _ACT_TABLE_PATCHED
    if _ACT_TABLE_PATCHED:
        return
    import concourse.bacc as bacc_mod
    from concourse.hw_specs import get_activation_tables as _orig
    target = "natural_log_exp_and_others"

    def patched(arch):
        tabs = _orig(arch)
        if target not in tabs:
            return tabs
        # The kernel's only activations are Relu/Exp/Ln, all in the target
        # set, so every other set can be hidden from the chooser.
        return {
            name: (funcs if name == target else set())
            for name, funcs in tabs.items()
        }

    bacc_mod.get_activation_tables = patched
    _ACT_TABLE_PATCHED = True


def build_program():
    _patch_act_tables()
    nc = bacc.Bacc()

    gx_d = nc.dram_tensor("gx", [P, W], BF16, kind="ExternalInput")
    # packed per-core constants (f32 words): [0:256] ln(wd) x2,
    # [256:384] (i+1) x2 as packed bf16 pairs, [384:386] negated chosen
    consts_d = nc.dram_tensor("consts", [P, W + S + TILES_PER_CORE], F32,
                              kind="ExternalInput")
    out_d = nc.dram_tensor("partial", [P, TILES_PER_CORE], F32,
                           kind="ExternalOutput")

    with tile.TileContext(nc) as tc, ExitStack() as ctx:
        const = ctx.enter_context(tc.tile_pool(name="const", bufs=1))
        big = ctx.enter_context(tc.tile_pool(name="big", bufs=3))
        work = ctx.enter_context(tc.tile_pool(name="work", bufs=12))
        cols = ctx.enter_context(tc.tile_pool(name="cols", bufs=6))
        fence_deps = []

        # ---- input DMAs, split across the two HWDGE queues ----
        A = big.tile([P, W], BF16, tag="A")
        fence_deps.append(nc.sync.dma_start(A[0:64, :], gx_d[0:64, :]))
        fence_deps.append(nc.scalar.dma_start(A[64:P, :], gx_d[64:P, :]))
        consts_sb = const.tile([P, W + S + TILES_PER_CORE], F32)
        fence_deps.append(nc.sync.dma_start(consts_sb[:], consts_d[:]))
        lnwd2 = consts_sb[:, 0:W]
        ip2 = consts_sb[:, W:W + S].bitcast(BF16)        # [P, 256] bf16
        nch = consts_sb[:, W + S:W + S + TILES_PER_CORE]

        zeros16 = const.tile([P, W], BF16)
        nc.gpsimd.memset(zeros16[:], 0.0)

        # dummy scan: absorbs the first-TTS warmup penalty during DMA wait
        dummy = cols.tile([P, 8], F32, name="dummy", tag="dummy")
        nc.vector.tensor_tensor_scan(
            out=dummy[:], data0=zeros16[:, 0:8], data1=zeros16[:, 0:8],
            initial=0.0, op0=Alu.add, op1=Alu.add)

        s2 = const.tile([P, TILES_PER_CORE], F32)
        raw2 = const.tile([P, TILES_PER_CORE], F32)
        act_insts = []

        # ---- s accumulation from the RAW (unsorted) input, overlapped with
        # the sort: s = sum relu(x - chosen) is order-independent.
        junk = work.tile([P, W], BF16, name="junk", tag="junk")
        for t in range(TILES_PER_CORE):
            act_insts.append(nc.scalar.activation(
                out=junk[:, t * S:(t + 1) * S], in_=A[:, t * S:(t + 1) * S],
                func=Act.Relu, bias=nch[:, t:t + 1],
                accum_out=s2[:, t:t + 1]))

        # ---- bitonic sort network: 28 stages x 2 DVE tensor_tensor ops ----
        Bt = big.tile([P, W], BF16, tag="B")
        Ct = big.tile([P, W], BF16, tag="C")
        pingpong = [Bt, Ct]
        k = 0
        src = A
        for L in (1, 2, 4, 8, 16, 32, 64):
            # merge stage: pair run A[i] with reversed run B (cols 2L-1-i)
            dst = pingpong[k % 2]
            k += 1
            nbt = S // (2 * L)
            if nbt > 1:
                vs = src[:].rearrange("p (t nb c) -> p t nb c",
                                      t=TILES_PER_CORE, nb=nbt, c=2 * L)
                vd = dst[:].rearrange("p (t nb c) -> p t nb c",
                                      t=TILES_PER_CORE, nb=nbt, c=2 * L)
                lo_i = vs[:, :, :, 0:L]
                hirev_i = vs[:, :, :, 2 * L - 1:L - 1:-1]
                lo_o = vd[:, :, :, 0:L]
                lorev_i = vs[:, :, :, L - 1::-1]
                hi_i = vs[:, :, :, L:2 * L]
                hi_o = vd[:, :, :, L:2 * L]
            else:
                vs = src[:].rearrange("p (nb c) -> p nb c",
                                      nb=W // (2 * L), c=2 * L)
                vd = dst[:].rearrange("p (nb c) -> p nb c",
                                      nb=W // (2 * L), c=2 * L)
                lo_i = vs[:, :, 0:L]
                hirev_i = vs[:, :, 2 * L - 1:L - 1:-1]
                lo_o = vd[:, :, 0:L]
                lorev_i = vs[:, :, L - 1::-1]
                hi_i = vs[:, :, L:2 * L]
                hi_o = vd[:, :, L:2 * L]
            nc.vector.tensor_tensor(out=lo_o, in0=lo_i, in1=hirev_i,
                                    op=Alu.max)
            nc.vector.tensor_tensor(out=hi_o, in0=lorev_i, in1=hi_i,
                                    op=Alu.min)
            src = dst
            d = L // 2
            while d >= 1:
                dst = pingpong[k % 2]
                k += 1
                vs = src[:].rearrange("p (nb c) -> p nb c",
                                      nb=W // (2 * d), c=2 * d)
                vd = dst[:].rearrange("p (nb c) -> p nb c",
                                      nb=W // (2 * d), c=2 * d)
                nc.vector.tensor_tensor(out=vd[:, :, 0:d], in0=vs[:, :, 0:d],
                                        in1=vs[:, :, d:2 * d], op=Alu.max)
                nc.vector.tensor_tensor(out=vd[:, :, d:2 * d],
                                        in0=vs[:, :, 0:d],
                                        in1=vs[:, :, d:2 * d], op=Alu.min)
                src = dst
                d //= 2
        D = src  # descending-sorted bf16, both tiles

        # ---- post-chain (no f32 cast; bf16 ins -> f32 outs) ----
        # fused prefix sum across both tiles; tile1's offset is corrected
        # via the per-partition scalar folded into its min() op below
        w1 = work.tile([P, W], F32, name="w1", tag="w1")
        nc.vector.tensor_tensor(out=w1[:], in0=D[:], in1=ip2, op=Alu.mult)
        ps = work.tile([P, W], F32, name="ps", tag="ps")
        nc.vector.tensor_tensor_scan(
            out=ps[:], data0=D[:], data1=zeros16[:], initial=0.0,
            op0=Alu.add, op1=Alu.add)
        # sc1 = s2[tile1] + ps[:, 127]  (boundary correction + s in one)
        sc1 = cols.tile([P, 1], F32, name="sc1", tag="sc1")
        nc.gpsimd.tensor_tensor(out=sc1[:], in0=s2[:, 1:2],
                                in1=ps[:, S - 1:S], op=Alu.add)
        ntd = work.tile([P, W], F32, name="ntd", tag="ntd")
        nc.vector.tensor_tensor(out=ntd[:], in0=w1[:], in1=ps[:],
                                op=Alu.subtract)
        # per tile: m' = min(ntd + s, 0); q = m' + ln wd; raw = sum exp(q)
        mprime = work.tile([P, W], F32, name="mprime", tag="mprime")
        q2 = work.tile([P, W], F32, name="q2", tag="q2")
        e2 = work.tile([P, W], F32, name="e2", tag="e2")
        svec = [s2[:, 0:1], sc1[:]]
        for t in range(TILES_PER_CORE):
            sl = slice(t * S, (t + 1) * S)
            nc.vector.tensor_scalar(
                out=mprime[:, sl], in0=ntd[:, sl], scalar1=svec[t],
                scalar2=0.0, op0=Alu.add, op1=Alu.min)
            nc.vector.tensor_tensor(
                out=q2[:, sl], in0=mprime[:, sl], in1=lnwd2[:, sl],
                op=Alu.add)
            act_insts.append(nc.scalar.activation(
                out=e2[:, sl], in_=q2[:, sl], func=Act.Exp,
                accum_out=raw2[:, t:t + 1]))

        # ln(1 - raw) in one ACT op, contrib = ln - s, one output DMA
        ln2 = cols.tile([P, TILES_PER_CORE], F32, name="ln2", tag="ln2")
        act_insts.append(nc.scalar.activation(
            out=ln2[:], in_=raw2[:], func=Act.Ln, scale=-1.0, bias=1.0))
        contrib2 = cols.tile([P, TILES_PER_CORE], F32, name="contrib2",
                             tag="contrib2")
        nc.gpsimd.tensor_tensor(out=contrib2[:], in0=ln2[:], in1=s2[:],
                                op=Alu.subtract)
        fence_deps.append(nc.sync.dma_start(out_d[:], contrib2[:]))

        # Staged SP fences: absorb per-proc completion sems a few at a time so
        # the kernel-tail Drain never carries more sync waits than the CTRL
        # instruction encoding allows.
        fence_deps.extend(act_insts[-2:])
        for i0 in range(0, len(fence_deps), 3):
            nop = nc.sync.nop()
            for dep in fence_deps[i0:i0 + 3]:
                tile.add_dep_helper(nop.ins, dep.ins, sync=True,
                                    reason="tail fence")

    nc.compile()
    return nc


def make_inputs(x, y, assortments):
    """Host-side sharding: per-core input maps (pure index/layout work)."""
    import ml_dtypes
    x = np.ascontiguousarray(np.asarray(x, dtype=np.float32).reshape(N))
    y = np.ascontiguousarray(np.asarray(y, dtype=np.float32).reshape(N))
    a = np.ascontiguousarray(np.asarray(assortments, dtype=np.int32).reshape(B, S))

    i = np.arange(S, dtype=np.float64)
    lnwd = np.full(S, -1.0e4, dtype=np.float32)
    lnwd[1:] = np.log(1.0 / (i[1:] * (i[1:] + 1.0))).astype(np.float32)
    lnwd2 = np.tile(lnwd, TILES_PER_CORE)
    ip2_words = np.ascontiguousarray(
        np.tile((i + 1.0).astype(ml_dtypes.bfloat16), TILES_PER_CORE)
    ).view(np.float32)  # 256 bf16 -> 128 f32 words

    in_maps = []
    for c in range(N_CORES):
        rows = a[c * ROWS_PER_CORE:(c + 1) * ROWS_PER_CORE]  # [256, 128]
        rs = np.sort(rows, axis=1)  # per-row item ids ascending (id-order shard)
        xv16 = x[rs].astype(ml_dtypes.bfloat16)       # [256, S] bf16 scores
        cidx = np.argmax(y[rs], axis=1)               # one-hot position per row
        cv = xv16[np.arange(ROWS_PER_CORE), cidx].astype(np.float32)
        gx = np.ascontiguousarray(
            xv16.reshape(TILES_PER_CORE, P, S).transpose(1, 0, 2)
            .reshape(P, W))
        nch = np.ascontiguousarray(-cv.reshape(TILES_PER_CORE, P).T)
        consts = np.ascontiguousarray(np.concatenate([
            np.tile(lnwd2[None, :], (P, 1)),
            np.tile(ip2_words[None, :], (P, 1)),
            nch,
        ], axis=1).astype(np.float32))
        in_maps.append({"gx": gx, "consts": consts})
    return in_maps


_PROGRAM_CACHE = {}


def kernel(x, y, assortments, _want_trace=False, _trace_kwargs=None):
    assert np.asarray(x).size == N and np.asarray(assortments).shape == (B, S)
    in_maps = make_inputs(x, y, assortments)
    if "nc" not in _PROGRAM_CACHE:
        _PROGRAM_CACHE["nc"] = build_program()
    nc = _PROGRAM_CACHE["nc"]
    res = run_bass_kernel_spmd(
        nc, in_maps, core_ids=list(range(N_CORES)),
        trace=_want_trace, **(_trace_kwargs or {})
    )
    partials = [np.asarray(res.results[c]["partial"]).reshape(-1).sum(dtype=np.float64) for c in range(N_CORES)]
    total = np.float32(np.sum(np.stack(partials), dtype=np.float64))
    out = np.float32(-total / np.float32(B))
    if _want_trace:
        return out, res
    return out
